# revision 1
# baseline (speedup 1.0000x reference)
import time
import numpy as np

# nn_CLSAEncoder: 2-layer ConvLSTM + causal softmax attention encoder.
# Shapes hardcoded per spec: B=16, T=256, ROWS=32, COLS=6, CH=16, D=3072.
ROWS, COLS, CH, KK, T = 32, 6, 16, 3, 256
D = ROWS * COLS * CH  # 3072
B = 16
NCORES = 8
RPC = T * B // NCORES  # 512 rows per core for the fused matmul

STATS = {"hw_ns": 0, "dispatch_wall_ns": 0}

_cache = {}


def _get_nc():
    """Build (once) the per-core Bass kernel: c[512, 6144] = hT.T @ w.

    hT: [D, RPC] fp32 (the core's slice of H, pre-transposed on host)
    w:  [D, 2D] fp32 (replicated fused weight [W1.T | W2.T])
    """
    if "nc" in _cache:
        return _cache["nc"]
    import concourse.bass as bass
    import concourse.bacc as bacc
    import concourse.mybir as mybir
    from concourse import tile

    dt = mybir.dt.float32
    nc = bacc.Bacc(None, target_bir_lowering=False)
    hT = nc.dram_tensor("hT", (D, RPC), dt, kind="ExternalInput")
    w = nc.dram_tensor("w", (D, 2 * D), dt, kind="ExternalInput")
    c = nc.dram_tensor("c", (RPC, 2 * D), dt, kind="ExternalOutput")

    KC = D // 128      # 24 contraction chunks
    NT = (2 * D) // 512  # 12 output column tiles
    MT = RPC // 128    # 4 output row tiles

    hT_v = hT.rearrange("(kc p) m -> p kc m", p=128)
    w_v = w.rearrange("(kc p) n -> p kc n", p=128)

    with tile.TileContext(nc) as tc:
        with (
            tc.tile_pool(name="hpool", bufs=1) as hpool,
            tc.tile_pool(name="wpool", bufs=3) as wpool,
            tc.tile_pool(name="opool", bufs=4) as opool,
            tc.tile_pool(name="psum", bufs=4, space=bass.MemorySpace.PSUM) as pspool,
        ):
            hsb = hpool.tile([128, KC, RPC], dt)
            nc.sync.dma_start(hsb[:], hT_v[:])
            for ns in range(NT):
                wsb = wpool.tile([128, KC, 512], dt)
                nc.sync.dma_start(wsb[:], w_v[:, :, ns * 512:(ns + 1) * 512])
                for m in range(MT):
                    ps = pspool.tile([128, 512], dt)
                    for k in range(KC):
                        nc.tensor.matmul(
                            ps[:],
                            hsb[:, k, m * 128:(m + 1) * 128],
                            wsb[:, k, :],
                            start=(k == 0),
                            stop=(k == KC - 1),
                        )
                    osb = opool.tile([128, 512], dt)
                    nc.vector.tensor_copy(osb[:], ps[:])
                    nc.sync.dma_start(
                        c[m * 128:(m + 1) * 128, ns * 512:(ns + 1) * 512], osb[:]
                    )
    nc.compile()
    _cache["nc"] = nc
    return nc


def _run_fuse(Hflat, Wcat):
    """Hflat: (T*B, D) fp32; Wcat: (D, 2D) fp32 -> (T*B, 2D) = Hflat @ Wcat."""
    from concourse.bass_utils import run_bass_kernel_spmd

    nc = _get_nc()
    in_maps = []
    for ci in range(NCORES):
        hT = np.ascontiguousarray(Hflat[ci * RPC:(ci + 1) * RPC].T)
        in_maps.append({"hT": hT, "w": Wcat})
    t0 = time.time()
    res = run_bass_kernel_spmd(nc, in_maps, core_ids=list(range(NCORES)))
    STATS["dispatch_wall_ns"] += int((time.time() - t0) * 1e9)
    if res.exec_time_ns:
        STATS["hw_ns"] += int(res.exec_time_ns)
    return np.concatenate([r["c"] for r in res.results], axis=0)


def _sig(x):
    return 1.0 / (1.0 + np.exp(-x))


def _conv(x, w, b):
    # x: (N, Cin, COLS), w: (O, Cin, 3) -> (N, O, COLS), SAME padding
    xp = np.pad(x, ((0, 0), (0, 0), (1, 1)))
    out = np.einsum("nic,oi->noc", xp[:, :, 0:COLS], w[:, :, 0])
    out += np.einsum("nic,oi->noc", xp[:, :, 1:COLS + 1], w[:, :, 1])
    out += np.einsum("nic,oi->noc", xp[:, :, 2:COLS + 2], w[:, :, 2])
    return out + b[None, :, None]


def _conv_lstm(frames, wx, bx, wh, bh):
    # frames: (T, B, ROWS, Cin, COLS) -> H: (T, B, D)
    N = B * ROWS
    h = np.zeros((N, CH, COLS), np.float32)
    c = np.zeros((N, CH, COLS), np.float32)
    H = np.empty((T, B, D), np.float32)
    for t in range(T):
        x = frames[t].reshape(N, -1, COLS)
        g = _conv(x, wx, bx) + _conv(h, wh, bh)
        i, f, o, gg = np.split(g, 4, axis=1)
        c = _sig(f) * c + _sig(i) * np.tanh(gg)
        h = _sig(o) * np.tanh(c)
        H[t] = h.reshape(B, D)
    return H


def _attn_scan(H, A, P):
    """H,A,P: (T,B,D). Returns refined (T,B,D) and attn weights (T,B,T)."""
    refB = np.zeros((B, T, D), np.float32)   # per-b contiguous history
    PB = np.ascontiguousarray(P.transpose(1, 0, 2))  # (B, T, D)
    Wmat = np.zeros((T, B, T), np.float32)
    refB[:, 0] = np.tanh(A[0])
    u = np.empty((B, D), np.float32)
    s_full = np.empty((B, T), np.float32)
    for t in range(1, T):
        s = s_full[:, :t]
        ht = H[t]
        for b in range(B):
            np.dot(refB[b, :t], ht[b], out=s[b])
        m = s.max(axis=1, keepdims=True)
        e = np.exp(s - m)
        wt = e / e.sum(axis=1, keepdims=True)
        Wmat[t, :, :t] = wt
        for b in range(B):
            np.dot(wt[b], PB[b, :t], out=u[b])
        refB[:, t] = np.tanh(A[t] + u)
    return np.ascontiguousarray(refB.transpose(1, 0, 2)), Wmat


def _layer(frames, wx, bx, wh, bh, Wf, bf):
    H = _conv_lstm(frames, wx, bx, wh, bh)  # (T, B, D)
    Wcat = np.ascontiguousarray(
        np.concatenate([Wf[:, :D].T, Wf[:, D:].T], axis=1), dtype=np.float32
    )
    C = _run_fuse(H.reshape(T * B, D), Wcat)  # (T*B, 2D)
    A = C[:, :D].reshape(T, B, D) + bf[None, None, :]
    P = np.ascontiguousarray(C[:, D:]).reshape(T, B, D)
    ref, Wmat = _attn_scan(H, A, P)
    return ref, Wmat


def kernel(x_flat, wx0, bx0, wh0, bh0, Wf0, bf0, wx1, bx1, wh1, bh1, Wf1, bf1):
    x_flat = np.asarray(x_flat, np.float32)
    frames0 = np.moveaxis(
        x_flat.reshape(B, T, ROWS, 1, COLS), 1, 0
    )  # (T, B, ROWS, 1, COLS)
    ref0, W0 = _layer(frames0, np.asarray(wx0), np.asarray(bx0),
                      np.asarray(wh0), np.asarray(bh0),
                      np.asarray(Wf0), np.asarray(bf0))
    frames1 = ref0.reshape(T, B, ROWS, CH, COLS)
    ref1, W1 = _layer(frames1, np.asarray(wx1), np.asarray(bx1),
                      np.asarray(wh1), np.asarray(bh1),
                      np.asarray(Wf1), np.asarray(bf1))
    enc = np.moveaxis(ref1, 0, 1)       # (B, T, D)
    a0 = np.moveaxis(W0, 0, 1)          # (B, T, T)
    a1 = np.moveaxis(W1, 0, 1)
    return enc, a0, a1


# revision 5
# speedup vs baseline: 4.7778x; 4.7778x over previous
import time
import numpy as np

# nn_CLSAEncoder: 2-layer ConvLSTM + causal softmax attention encoder.
# Shapes hardcoded per spec: B=16, T=256, ROWS=32, COLS=6, CH=16, D=3072.
ROWS, COLS, CH, KK, T = 32, 6, 16, 3, 256
D = ROWS * COLS * CH  # 3072
B = 16
NCORES = 8
RPC = T * B // NCORES  # 512 rows per core for the fused matmul

STATS = {"hw_ns": 0, "dispatch_wall_ns": 0}

_cache = {}


def _get_nc():
    """Build (once) the per-core Bass kernel: c[512, 6144] = hT.T @ w.

    hT: [D, RPC] fp32 (the core's slice of H, pre-transposed on host)
    w:  [D, 2D] fp32 (replicated fused weight [W1.T | W2.T])
    """
    if "nc" in _cache:
        return _cache["nc"]
    import concourse.bass as bass
    import concourse.bacc as bacc
    import concourse.mybir as mybir
    from concourse import tile

    dt = mybir.dt.float32r  # fp32 bits, replicated-mode matmul (4x faster, N>=256)
    dto = mybir.dt.float32
    nc = bacc.Bacc(None, target_bir_lowering=False)
    hT = nc.dram_tensor("hT", (D, RPC), dt, kind="ExternalInput")
    w = nc.dram_tensor("w", (D, 2 * D), dt, kind="ExternalInput")
    c = nc.dram_tensor("c", (RPC, 2 * D), dto, kind="ExternalOutput")

    KC = D // 128      # 24 contraction chunks
    NT = (2 * D) // 512  # 12 output column tiles
    MT = RPC // 128    # 4 output row tiles

    hT_v = hT.rearrange("(kc p) m -> p kc m", p=128)
    w_v = w.rearrange("(kc p) n -> p kc n", p=128)

    with tile.TileContext(nc) as tc:
        with (
            tc.tile_pool(name="hpool", bufs=1) as hpool,
            tc.tile_pool(name="wpool", bufs=3) as wpool,
            tc.tile_pool(name="opool", bufs=4) as opool,
            tc.tile_pool(name="psum", bufs=4, space=bass.MemorySpace.PSUM) as pspool,
        ):
            hsb = hpool.tile([128, KC, RPC], dt)
            nc.sync.dma_start(hsb[:], hT_v[:])
            for ns in range(NT):
                wsb = wpool.tile([128, KC, 512], dt)
                nc.sync.dma_start(wsb[:], w_v[:, :, ns * 512:(ns + 1) * 512])
                for m in range(MT):
                    ps = pspool.tile([128, 512], dto)
                    for k in range(KC):
                        nc.tensor.matmul(
                            ps[:],
                            hsb[:, k, m * 128:(m + 1) * 128],
                            wsb[:, k, :],
                            start=(k == 0),
                            stop=(k == KC - 1),
                        )
                    osb = opool.tile([128, 512], dto)
                    nc.vector.tensor_copy(osb[:], ps[:])
                    nc.sync.dma_start(
                        c[m * 128:(m + 1) * 128, ns * 512:(ns + 1) * 512], osb[:]
                    )
    nc.compile()
    _cache["nc"] = nc
    return nc


def _run_fuse(Hflat, Wcat):
    """Hflat: (T*B, D) fp32; Wcat: (D, 2D) fp32 -> (T*B, 2D) = Hflat @ Wcat."""
    from concourse.bass_utils import run_bass_kernel_spmd

    nc = _get_nc()
    in_maps = []
    for ci in range(NCORES):
        hT = np.ascontiguousarray(Hflat[ci * RPC:(ci + 1) * RPC].T)
        in_maps.append({"hT": hT, "w": Wcat})
    t0 = time.time()
    res = run_bass_kernel_spmd(nc, in_maps, core_ids=list(range(NCORES)))
    dt_ns = int((time.time() - t0) * 1e9)
    STATS["dispatch_wall_ns"] += dt_ns
    STATS.setdefault("calls", []).append(dt_ns)
    if res.exec_time_ns:
        STATS["hw_ns"] += int(res.exec_time_ns)
    return np.concatenate([r["c"] for r in res.results], axis=0)


def _sig(x):
    return 1.0 / (1.0 + np.exp(-x))


def _conv(x, w, b):
    # x: (N, Cin, COLS), w: (O, Cin, 3) -> (N, O, COLS), SAME padding
    xp = np.pad(x, ((0, 0), (0, 0), (1, 1)))
    out = np.einsum("nic,oi->noc", xp[:, :, 0:COLS], w[:, :, 0])
    out += np.einsum("nic,oi->noc", xp[:, :, 1:COLS + 1], w[:, :, 1])
    out += np.einsum("nic,oi->noc", xp[:, :, 2:COLS + 2], w[:, :, 2])
    return out + b[None, :, None]


def _conv_lstm(frames, wx, bx, wh, bh):
    # frames: (T, B, ROWS, Cin, COLS) -> H: (T, B, D)
    N = B * ROWS
    h = np.zeros((N, CH, COLS), np.float32)
    c = np.zeros((N, CH, COLS), np.float32)
    H = np.empty((T, B, D), np.float32)
    for t in range(T):
        x = frames[t].reshape(N, -1, COLS)
        g = _conv(x, wx, bx) + _conv(h, wh, bh)
        i, f, o, gg = np.split(g, 4, axis=1)
        c = _sig(f) * c + _sig(i) * np.tanh(gg)
        h = _sig(o) * np.tanh(c)
        H[t] = h.reshape(B, D)
    return H


def _attn_scan(H, A, P):
    """H,A,P: (T,B,D). Returns refined (T,B,D) and attn weights (T,B,T)."""
    refB = np.zeros((B, T, D), np.float32)   # per-b contiguous history
    PB = np.ascontiguousarray(P.transpose(1, 0, 2))  # (B, T, D)
    Wmat = np.zeros((T, B, T), np.float32)
    refB[:, 0] = np.tanh(A[0])
    u = np.empty((B, D), np.float32)
    s_full = np.empty((B, T), np.float32)
    for t in range(1, T):
        s = s_full[:, :t]
        ht = H[t]
        for b in range(B):
            np.dot(refB[b, :t], ht[b], out=s[b])
        m = s.max(axis=1, keepdims=True)
        e = np.exp(s - m)
        wt = e / e.sum(axis=1, keepdims=True)
        Wmat[t, :, :t] = wt
        for b in range(B):
            np.dot(wt[b], PB[b, :t], out=u[b])
        refB[:, t] = np.tanh(A[t] + u)
    return np.ascontiguousarray(refB.transpose(1, 0, 2)), Wmat


def _layer(frames, wx, bx, wh, bh, Wf, bf):
    H = _conv_lstm(frames, wx, bx, wh, bh)  # (T, B, D)
    Wcat = np.ascontiguousarray(
        np.concatenate([Wf[:, :D].T, Wf[:, D:].T], axis=1), dtype=np.float32
    )
    C = _run_fuse(H.reshape(T * B, D), Wcat)  # (T*B, 2D)
    A = C[:, :D].reshape(T, B, D) + bf[None, None, :]
    P = np.ascontiguousarray(C[:, D:]).reshape(T, B, D)
    ref, Wmat = _attn_scan(H, A, P)
    return ref, Wmat


def kernel(x_flat, wx0, bx0, wh0, bh0, Wf0, bf0, wx1, bx1, wh1, bh1, Wf1, bf1):
    x_flat = np.asarray(x_flat, np.float32)
    frames0 = np.moveaxis(
        x_flat.reshape(B, T, ROWS, 1, COLS), 1, 0
    )  # (T, B, ROWS, 1, COLS)
    ref0, W0 = _layer(frames0, np.asarray(wx0), np.asarray(bx0),
                      np.asarray(wh0), np.asarray(bh0),
                      np.asarray(Wf0), np.asarray(bf0))
    frames1 = ref0.reshape(T, B, ROWS, CH, COLS)
    ref1, W1 = _layer(frames1, np.asarray(wx1), np.asarray(bx1),
                      np.asarray(wh1), np.asarray(bh1),
                      np.asarray(Wf1), np.asarray(bf1))
    enc = np.moveaxis(ref1, 0, 1)       # (B, T, D)
    a0 = np.moveaxis(W0, 0, 1)          # (B, T, T)
    a1 = np.moveaxis(W1, 0, 1)
    return enc, a0, a1


# revision 8
# speedup vs baseline: 6.3503x; 1.3291x over previous
import time
import numpy as np

# nn_CLSAEncoder: 2-layer ConvLSTM + causal softmax attention encoder.
# Shapes hardcoded per spec: B=16, T=256, ROWS=32, COLS=6, CH=16, D=3072.
ROWS, COLS, CH, KK, T = 32, 6, 16, 3, 256
D = ROWS * COLS * CH  # 3072
B = 16
NCORES = 8
RSH, CSH = 2, 4                # core grid: 2-way rows x 4-way cols
RPC = T * B // RSH             # 2048 rows per core
CPC = 2 * D // CSH             # 1536 output cols per core

STATS = {"hw_ns": 0, "dispatch_wall_ns": 0}

_cache = {}


def _get_nc():
    """Build (once) the per-core Bass kernel: c[512, 6144] = hT.T @ w.

    hT: [D, RPC] fp32 (the core's slice of H, pre-transposed on host)
    w:  [D, 2D] fp32 (replicated fused weight [W1.T | W2.T])
    """
    if "nc" in _cache:
        return _cache["nc"]
    import concourse.bass as bass
    import concourse.bacc as bacc
    import concourse.mybir as mybir
    from concourse import tile

    dt = mybir.dt.float32r  # fp32 bits, replicated-mode matmul (4x faster, N>=256)
    dto = mybir.dt.float32
    nc = bacc.Bacc(None, target_bir_lowering=False)
    hT = nc.dram_tensor("hT", (D, RPC), dt, kind="ExternalInput")
    w = nc.dram_tensor("w", (D, CPC), dt, kind="ExternalInput")
    c = nc.dram_tensor("c", (RPC, CPC), dto, kind="ExternalOutput")

    KC = D // 128     # 24 contraction chunks
    NT = CPC // 512   # 3 output column tiles
    MT = RPC // 128   # 16 output row tiles

    hT_v = hT.rearrange("(kc p) m -> p kc m", p=128)
    w_v = w.rearrange("(kc p) n -> p kc n", p=128)

    with tile.TileContext(nc) as tc:
        with (
            tc.tile_pool(name="hpool", bufs=3) as hpool,
            tc.tile_pool(name="wpool", bufs=1) as wpool,
            tc.tile_pool(name="opool", bufs=4) as opool,
            tc.tile_pool(name="psum", bufs=4, space=bass.MemorySpace.PSUM) as pspool,
        ):
            wsb = wpool.tile([128, KC, CPC], dt)
            nc.sync.dma_start(wsb[:], w_v[:])
            for m in range(MT):
                hm = hpool.tile([128, KC, 128], dt)
                nc.sync.dma_start(hm[:], hT_v[:, :, m * 128:(m + 1) * 128])
                for ns in range(NT):
                    ps = pspool.tile([128, 512], dto)
                    for k in range(KC):
                        nc.tensor.matmul(
                            ps[:],
                            hm[:, k, :],
                            wsb[:, k, ns * 512:(ns + 1) * 512],
                            start=(k == 0),
                            stop=(k == KC - 1),
                        )
                    osb = opool.tile([128, 512], dto)
                    nc.vector.tensor_copy(osb[:], ps[:])
                    nc.sync.dma_start(
                        c[m * 128:(m + 1) * 128, ns * 512:(ns + 1) * 512], osb[:]
                    )
    nc.compile()
    _cache["nc"] = nc
    return nc


def _run_fuse(Hflat, Wcat):
    """Hflat: (T*B, D) fp32; Wcat: (D, 2D) fp32 -> (T*B, 2D) = Hflat @ Wcat."""
    from concourse.bass_utils import run_bass_kernel_spmd

    nc = _get_nc()
    in_maps = []
    hT_parts = [
        np.ascontiguousarray(Hflat[ri * RPC:(ri + 1) * RPC].T) for ri in range(RSH)
    ]
    w_parts = [
        np.ascontiguousarray(Wcat[:, cj * CPC:(cj + 1) * CPC]) for cj in range(CSH)
    ]
    for ci in range(NCORES):
        in_maps.append({"hT": hT_parts[ci // CSH], "w": w_parts[ci % CSH]})
    t0 = time.time()
    res = run_bass_kernel_spmd(nc, in_maps, core_ids=list(range(NCORES)))
    dt_ns = int((time.time() - t0) * 1e9)
    STATS["dispatch_wall_ns"] += dt_ns
    STATS.setdefault("calls", []).append(dt_ns)
    if res.exec_time_ns:
        STATS["hw_ns"] += int(res.exec_time_ns)
    C = np.empty((T * B, 2 * D), np.float32)
    for ci in range(NCORES):
        ri, cj = ci // CSH, ci % CSH
        C[ri * RPC:(ri + 1) * RPC, cj * CPC:(cj + 1) * CPC] = res.results[ci]["c"]
    return C


def _sig(x):
    return 1.0 / (1.0 + np.exp(-x))


def _conv(x, w, b):
    # x: (N, Cin, COLS), w: (O, Cin, 3) -> (N, O, COLS), SAME padding
    xp = np.pad(x, ((0, 0), (0, 0), (1, 1)))
    out = np.einsum("nic,oi->noc", xp[:, :, 0:COLS], w[:, :, 0])
    out += np.einsum("nic,oi->noc", xp[:, :, 1:COLS + 1], w[:, :, 1])
    out += np.einsum("nic,oi->noc", xp[:, :, 2:COLS + 2], w[:, :, 2])
    return out + b[None, :, None]


def _conv_lstm(frames, wx, bx, wh, bh):
    # frames: (T, B, ROWS, Cin, COLS) -> H: (T, B, D)
    N = B * ROWS
    h = np.zeros((N, CH, COLS), np.float32)
    c = np.zeros((N, CH, COLS), np.float32)
    H = np.empty((T, B, D), np.float32)
    for t in range(T):
        x = frames[t].reshape(N, -1, COLS)
        g = _conv(x, wx, bx) + _conv(h, wh, bh)
        i, f, o, gg = np.split(g, 4, axis=1)
        c = _sig(f) * c + _sig(i) * np.tanh(gg)
        h = _sig(o) * np.tanh(c)
        H[t] = h.reshape(B, D)
    return H


def _attn_scan(H, A, P):
    """H,A,P: (T,B,D). Returns refined (T,B,D) and attn weights (T,B,T)."""
    refB = np.zeros((B, T, D), np.float32)   # per-b contiguous history
    PB = np.ascontiguousarray(P.transpose(1, 0, 2))  # (B, T, D)
    Wmat = np.zeros((T, B, T), np.float32)
    refB[:, 0] = np.tanh(A[0])
    u = np.empty((B, D), np.float32)
    s_full = np.empty((B, T), np.float32)
    for t in range(1, T):
        s = s_full[:, :t]
        ht = H[t]
        for b in range(B):
            np.dot(refB[b, :t], ht[b], out=s[b])
        m = s.max(axis=1, keepdims=True)
        e = np.exp(s - m)
        wt = e / e.sum(axis=1, keepdims=True)
        Wmat[t, :, :t] = wt
        for b in range(B):
            np.dot(wt[b], PB[b, :t], out=u[b])
        refB[:, t] = np.tanh(A[t] + u)
    return np.ascontiguousarray(refB.transpose(1, 0, 2)), Wmat


def _layer(frames, wx, bx, wh, bh, Wf, bf):
    H = _conv_lstm(frames, wx, bx, wh, bh)  # (T, B, D)
    Wcat = np.ascontiguousarray(
        np.concatenate([Wf[:, :D].T, Wf[:, D:].T], axis=1), dtype=np.float32
    )
    C = _run_fuse(H.reshape(T * B, D), Wcat)  # (T*B, 2D)
    A = C[:, :D].reshape(T, B, D) + bf[None, None, :]
    P = np.ascontiguousarray(C[:, D:]).reshape(T, B, D)
    ref, Wmat = _attn_scan(H, A, P)
    return ref, Wmat


def kernel(x_flat, wx0, bx0, wh0, bh0, Wf0, bf0, wx1, bx1, wh1, bh1, Wf1, bf1):
    x_flat = np.asarray(x_flat, np.float32)
    frames0 = np.moveaxis(
        x_flat.reshape(B, T, ROWS, 1, COLS), 1, 0
    )  # (T, B, ROWS, 1, COLS)
    ref0, W0 = _layer(frames0, np.asarray(wx0), np.asarray(bx0),
                      np.asarray(wh0), np.asarray(bh0),
                      np.asarray(Wf0), np.asarray(bf0))
    frames1 = ref0.reshape(T, B, ROWS, CH, COLS)
    ref1, W1 = _layer(frames1, np.asarray(wx1), np.asarray(bx1),
                      np.asarray(wh1), np.asarray(bh1),
                      np.asarray(Wf1), np.asarray(bf1))
    enc = np.moveaxis(ref1, 0, 1)       # (B, T, D)
    a0 = np.moveaxis(W0, 0, 1)          # (B, T, T)
    a1 = np.moveaxis(W1, 0, 1)
    return enc, a0, a1
